# revision 9
# baseline (speedup 1.0000x reference)
"""Causal self-attention Trainium2 Bass kernel.

Problem: nn_CausalSelfAttention (B=2, T=4096, C=512, H=8 heads, hd=64), fp32.

Sharding (8 cores): core c handles batch b = c//4 and head-pair hp = c%4
(heads 2*hp, 2*hp+1 -> a combined 128-wide head-dim slice D).  Each core:
  qT/kT/vT = (W[,D-slice] @ x_b^T) + bias          [128, T]   (d on partitions)
  v_nat    = vT^T per 128-key chunk                [T, 128]   (keys on partitions)
  scores^T s[k, q] = sum_d kT[d,k] qT[d,q]         (PE, row-packed per head)
  p = exp(s/8)  (no max subtraction; scores ~ N(0,1), fp32-safe)
  o^T[d, q] = sum_k v_nat[k, d] p[k, q]  (PE, col-packed heads, PSUM accum)
  den[q]    = sum_k p[k, q]              (PE, ones lhsT, broadcast 64 rows)
  oT = o^T / den ;  partial^T[j, t] = sum_d WpT[d, j] oT[d, t]  -> DRAM
Host: out[b] = (sum of the 4 per-core partials)^T + bp.
"""

import math
from functools import lru_cache

import numpy as np

N_EMBD = 512
N_HEAD = 8
HEAD_DIM = N_EMBD // N_HEAD  # 64
B, T = 2, 4096
N_CORES = 8
D = 128          # per-core head-dim slice (2 heads x 64)
NQ = 512         # query block
KC = 128         # key chunk (PE contraction)


@lru_cache(maxsize=None)
def build_nc(t_len=T, c_embd=N_EMBD, nq=NQ):
    import concourse.mybir as mybir
    import concourse.tile as tile
    from concourse import bacc
    from concourse.masks import make_identity

    f32 = mybir.dt.float32
    NCc = c_embd // 128          # c-chunks for the projections (4)
    NT = t_len // nq             # t/q blocks (8)
    NKC = t_len // KC            # key chunks (32)
    SUBS = nq // KC              # key chunks per q-block-width (4)

    nc = bacc.Bacc(
        "TRN2",
        target_bir_lowering=False,
        debug=False,
        enable_asserts=False,
        num_devices=N_CORES,
    )

    xT_d = nc.dram_tensor("xT", [c_embd, t_len], f32, kind="ExternalInput")
    wqT_d = nc.dram_tensor("wqT", [c_embd, D], f32, kind="ExternalInput")
    wkT_d = nc.dram_tensor("wkT", [c_embd, D], f32, kind="ExternalInput")
    wvT_d = nc.dram_tensor("wvT", [c_embd, D], f32, kind="ExternalInput")
    wpT_d = nc.dram_tensor("wpT", [D, c_embd], f32, kind="ExternalInput")
    bq_d = nc.dram_tensor("bq", [D, 1], f32, kind="ExternalInput")
    bk_d = nc.dram_tensor("bk", [D, 1], f32, kind="ExternalInput")
    bv_d = nc.dram_tensor("bv", [D, 1], f32, kind="ExternalInput")
    mask_d = nc.dram_tensor("mask", [128, 2 * 2 * nq], f32, kind="ExternalInput")
    outT_d = nc.dram_tensor("outT", [c_embd, t_len], f32, kind="ExternalOutput")

    scale = 1.0 / math.sqrt(HEAD_DIM)

    with tile.TileContext(nc) as tc:
        with (
            tc.tile_pool(name="consts", bufs=1) as consts,
            tc.tile_pool(name="xpool", bufs=1) as xpool,
            tc.tile_pool(name="qkv", bufs=1) as qkv,
            tc.tile_pool(name="vnat", bufs=1) as vnat_pool,
            tc.tile_pool(name="ppool", bufs=2) as ppool,
            tc.tile_pool(name="opool", bufs=1) as opool,
            tc.tile_pool(name="rpool", bufs=2) as rpool,
            tc.tile_pool(name="stage", bufs=4) as stage,
            tc.tile_pool(name="ps_sT", bufs=2, space="PSUM") as ps_sT,
            tc.tile_pool(name="ps_o", bufs=1, space="PSUM") as ps_o,
            tc.tile_pool(name="ps_den", bufs=1, space="PSUM") as ps_den,
            tc.tile_pool(name="ps_misc", bufs=2, space="PSUM") as ps_misc,
        ):
            # ---- constants ----
            wq_sb = consts.tile([128, NCc * D], f32)
            wk_sb = consts.tile([128, NCc * D], f32)
            wv_sb = consts.tile([128, NCc * D], f32)
            for c in range(NCc):
                nc.sync.dma_start(wq_sb[:, c * D:(c + 1) * D], wqT_d.ap()[c * 128:(c + 1) * 128, :])
                nc.sync.dma_start(wk_sb[:, c * D:(c + 1) * D], wkT_d.ap()[c * 128:(c + 1) * 128, :])
                nc.sync.dma_start(wv_sb[:, c * D:(c + 1) * D], wvT_d.ap()[c * 128:(c + 1) * 128, :])
            wp_sb = consts.tile([128, c_embd], f32)
            nc.sync.dma_start(wp_sb, wpT_d.ap())
            bq_sb = consts.tile([128, 1], f32)
            bk_sb = consts.tile([128, 1], f32)
            bv_sb = consts.tile([128, 1], f32)
            nc.sync.dma_start(bq_sb, bq_d.ap())
            nc.sync.dma_start(bk_sb, bk_d.ap())
            nc.sync.dma_start(bv_sb, bv_d.ap())
            mask_sb = consts.tile([128, 2 * 2 * nq], f32)
            nc.sync.dma_start(mask_sb, mask_d.ap())
            ident = consts.tile([128, 128], f32)
            make_identity(nc, ident)

            # ---- x (transposed) : [128, c, t] flattened 2D ----
            x_sb = xpool.tile([128, NCc * t_len], f32)
            for tq in range(NT):
                for c in range(NCc):
                    nc.sync.dma_start(
                        x_sb[:, c * t_len + tq * nq: c * t_len + (tq + 1) * nq],
                        xT_d.ap()[c * 128:(c + 1) * 128, tq * nq:(tq + 1) * nq],
                    )

            qT_s = qkv.tile([128, t_len], f32)
            kT_s = qkv.tile([128, t_len], f32)
            vT_s = qkv.tile([128, t_len], f32)
            # per key chunk kc: [v_A(64) | ones(64) | v_B(64) | ones(64)] at cols 256*kc
            v_nat = vnat_pool.tile([128, (t_len // KC) * 256], f32)
            oT_sb = qkv.tile([128, t_len], f32)
            vn4 = v_nat.rearrange("p (k g) -> p k g", g=256)
            nc.vector.memset(vn4[:, :, 64:128], 1.0)
            nc.vector.memset(vn4[:, :, 192:256], 1.0)

            for tb in range(NT):
                ts_ = slice(tb * nq, (tb + 1) * nq)

                # ---- QKV projection for this t-block ----
                for w_sb, b_sb, dst in ((wq_sb, bq_sb, qT_s), (wk_sb, bk_sb, kT_s), (wv_sb, bv_sb, vT_s)):
                    ps = ps_sT.tile([128, nq], f32, tag="sT")
                    for c in range(NCc):
                        nc.tensor.matmul(
                            ps,
                            lhsT=w_sb[:, c * D:(c + 1) * D],
                            rhs=x_sb[:, c * t_len + tb * nq: c * t_len + (tb + 1) * nq],
                            start=(c == 0),
                            stop=(c == NCc - 1),
                        )
                    nc.vector.tensor_scalar_add(dst[:, ts_], ps, b_sb)

                # ---- v transpose for this t-block's key chunks ----
                for sub in range(SUBS):
                    kc = tb * SUBS + sub
                    pst = ps_misc.tile([128, 128], f32, tag="misc")
                    nc.tensor.transpose(pst, vT_s[:, kc * KC:(kc + 1) * KC], ident)
                    nc.vector.tensor_copy(v_nat[:, kc * 256:kc * 256 + 64], pst[:, 0:64])
                    nc.vector.tensor_copy(v_nat[:, kc * 256 + 128:kc * 256 + 192], pst[:, 64:128])

                # ---- attention for q-block tb ----
                # bankA/bankB: rows 0-63 = head o accumulation, rows 64-127 = den (x64)
                bankA = ps_o.tile([128, nq], f32, tag="o")
                bankB = ps_den.tile([128, nq], f32, tag="den")
                nsup = (tb + 1) * SUBS // 2  # supers of 2 key chunks
                nkc = (tb + 1) * SUBS        # total key chunks for this block
                for sp in range(nsup):
                    sT_A = ps_sT.tile([128, 2 * nq], f32, tag="sT")
                    sT_B = ps_sT.tile([128, 2 * nq], f32, tag="sT")
                    p_AB = ppool.tile([128, 4 * nq], f32, tag="p")
                    for sub in range(2):
                        kc = 2 * sp + sub
                        ks = slice(kc * KC, (kc + 1) * KC)
                        nc.tensor.matmul(
                            sT_A[:, sub * nq:(sub + 1) * nq],
                            lhsT=kT_s[0:64, ks], rhs=qT_s[0:64, ts_],
                            start=True, stop=True,
                        )
                        nc.tensor.matmul(
                            sT_B[:, sub * nq:(sub + 1) * nq],
                            lhsT=kT_s[64:128, ks], rhs=qT_s[64:128, ts_],
                            start=True, stop=True,
                        )
                    nc.scalar.activation(
                        p_AB[:, 0:2 * nq], sT_A, mybir.ActivationFunctionType.Exp, scale=scale,
                    )
                    nc.scalar.activation(
                        p_AB[:, 2 * nq:4 * nq], sT_B, mybir.ActivationFunctionType.Exp, scale=scale,
                    )
                    dsp = sp - (nsup - 2)  # 0 or 1 for the two diagonal supers
                    if dsp >= 0:
                        ms = slice(dsp * 2 * nq, (dsp + 1) * 2 * nq)
                        nc.vector.tensor_mul(p_AB[:, 0:2 * nq], p_AB[:, 0:2 * nq], mask_sb[:, ms])
                        nc.vector.tensor_mul(p_AB[:, 2 * nq:4 * nq], p_AB[:, 2 * nq:4 * nq], mask_sb[:, ms])
                    for sub in range(2):
                        kc = 2 * sp + sub
                        first = kc == 0
                        last = kc == nkc - 1
                        pa = p_AB[:, sub * nq:(sub + 1) * nq]
                        pb = p_AB[:, (2 + sub) * nq:(3 + sub) * nq]
                        # lhsT = [v_h | ones64]: rows 0-63 accumulate o_h, rows
                        # 64-127 accumulate den_h (x64). One exclusive bank/group.
                        nc.tensor.matmul(bankA, lhsT=v_nat[:, kc * 256:kc * 256 + 128],
                                         rhs=pa, start=first, stop=last)
                        nc.tensor.matmul(bankB, lhsT=v_nat[:, kc * 256 + 128:kc * 256 + 256],
                                         rhs=pb, start=first, stop=last)

                # ---- normalize: oT = o / den ----
                r = rpool.tile([128, nq], f32, tag="r")
                nc.vector.reciprocal(r[0:64, :], bankA[64:128, :])
                nc.vector.reciprocal(r[64:128, :], bankB[64:128, :])
                nc.vector.tensor_mul(oT_sb[0:64, ts_], bankA[0:64, :], r[0:64, :])
                nc.vector.tensor_mul(oT_sb[64:128, ts_], bankB[0:64, :], r[64:128, :])

                # ---- output projection for this t-block ----
                for jc in range(NCc):
                    po = ps_misc.tile([128, nq], f32, tag="misc")
                    nc.tensor.matmul(
                        po, lhsT=wp_sb[:, jc * 128:(jc + 1) * 128], rhs=oT_sb[:, ts_],
                        start=True, stop=True,
                    )
                    st = stage.tile([128, nq], f32, tag="st")
                    nc.vector.tensor_copy(st, po)
                    nc.sync.dma_start(outT_d.ap()[jc * 128:(jc + 1) * 128, ts_], st)

    nc.compile()
    return nc


def make_mask(nq=NQ):
    # mask[k, m, sub, q] = 1.0 iff k + 128*(2m+sub) <= q  (diagonal-super masks)
    k = np.arange(128)[:, None, None, None]
    m = np.arange(2)[None, :, None, None]
    sub = np.arange(2)[None, None, :, None]
    q = np.arange(nq)[None, None, None, :]
    mask = (k + KC * (2 * m + sub) <= q).astype(np.float32)
    return mask.reshape(128, 2 * 2 * nq)


def make_in_maps(x, Wq, bq, Wk, bk, Wv, bv, Wp, t_len=T, c_embd=N_EMBD, nq=NQ):
    """Per-core input dicts. x: [B, t, C]; weights [C, C]; biases [C]."""
    mask = make_mask(nq)
    xT = [np.ascontiguousarray(x[b].T).astype(np.float32) for b in range(x.shape[0])]
    WqT, WkT, WvT, WpT = (np.ascontiguousarray(W.T).astype(np.float32) for W in (Wq, Wk, Wv, Wp))
    in_maps = []
    n_pairs = c_embd // D  # head-pairs (4)
    for core in range(N_CORES):
        b = core // n_pairs
        hp = core % n_pairs
        ds_ = slice(hp * D, (hp + 1) * D)
        in_maps.append({
            "xT": xT[b],
            "wqT": np.ascontiguousarray(WqT[:, ds_]),
            "wkT": np.ascontiguousarray(WkT[:, ds_]),
            "wvT": np.ascontiguousarray(WvT[:, ds_]),
            "wpT": np.ascontiguousarray(WpT[ds_, :]),
            "bq": np.ascontiguousarray(bq[ds_].reshape(D, 1)).astype(np.float32),
            "bk": np.ascontiguousarray(bk[ds_].reshape(D, 1)).astype(np.float32),
            "bv": np.ascontiguousarray(bv[ds_].reshape(D, 1)).astype(np.float32),
            "mask": mask,
        })
    return in_maps


def assemble_output(results, bp, t_len=T, c_embd=N_EMBD):
    """results: list of 8 dicts with 'outT' [C, t]. Returns [B, t, C]."""
    n_pairs = c_embd // D
    out = np.empty((B, t_len, c_embd), dtype=np.float32)
    for b in range(B):
        acc = np.zeros((c_embd, t_len), dtype=np.float32)
        for hp in range(n_pairs):
            acc += results[b * n_pairs + hp]["outT"]
        out[b] = acc.T + bp[None, :]
    return out


def kernel(x, weight, state, Wq, bq, Wk, bk, Wv, bv, Wp, bp, **_unused):
    from concourse.bass_utils import run_bass_kernel_spmd

    x = np.asarray(x, dtype=np.float32)
    Wq, bq, Wk, bk, Wv, bv, Wp, bp = (
        np.asarray(a, dtype=np.float32) for a in (Wq, bq, Wk, bk, Wv, bv, Wp, bp)
    )
    nc = build_nc()
    in_maps = make_in_maps(x, Wq, bq, Wk, bk, Wv, bv, Wp)
    res = run_bass_kernel_spmd(nc, in_maps, core_ids=list(range(N_CORES)))
    return assemble_output(res.results, bp)


if __name__ == "__main__":
    # smoke test of program construction only
    nc = build_nc()
    print("built ok")
